# revision 1
# baseline (speedup 1.0000x reference)
"""MLA attention (DeepSeek-style) distributed over 8 TRN2 NeuronCores.

Sharding: core c -> batch b=c//4, head-group/seq-shard g=c%4.
Phase A: down-proj of own 512-pos shard -> partition-major bf16 bounce
buffers -> 8-core shared-output AllGathers (kv first, then q in two
halves so comms overlap projection compute).
Phase B: up-proj (Qt/Kt d-major, V row-major), flash-style causal
attention with St[kv,q] layout. Softmax denominators via a bf16 P_sum
vector-accumulate + one ones-matmul per (head, q-chunk); the causal mask
is folded into the score-matmul PSUM group (identity stationary x wide
mask moving). Attention outputs AllGather (shared output) per head; the
row-parallel wo matmul on the own s-shard is interleaved between
attention heads and accumulated in SBUF.
"""

import numpy as np
import ml_dtypes

import concourse.bass as bass
import concourse.bacc as bacc
import concourse.tile as tile
import concourse.mybir as mybir
from concourse.bass_utils import run_bass_kernel_spmd

BF16 = ml_dtypes.bfloat16

# problem constants (hardcoded per harness rules)
DIM = 2048
N_HEADS = 16
Q_LORA = 1536
KV_LORA = 512
NOPE = 128
ROPE = 64
V_DIM = 128
QK_HD = NOPE + ROPE  # 192
EPS = 1e-6
B, S = 2, 2048
SCALE = QK_HD ** -0.5

NCORES = 8
GROUP = 4               # cores per batch
SSH = S // GROUP        # 512, seq shard
HPC = N_HEADS // GROUP  # 4 heads per core
P = 128
NKT = DIM // P          # 16
NQM = Q_LORA // P       # 12
NQ1 = 6                 # q slabs in first AG
NQ2 = NQM - NQ1         # 6
NKVM = KV_LORA // P     # 4
NCH = S // 512          # 4
# partition-major bounce widths (cols); each AG has a ~15us floor and
# blocks the gpsimd queue for its duration, so fewer/bigger AGs win
BKV_W = NKVM * SSH + SSH + SSH        # 3072: kv latent | rope dup | a_kv bc
BQ1_W = NQ1 * SSH                     # 3072
BQ2_W = NQ2 * SSH + SSH               # 3584: q slabs 6..11 | a_q bc
QF8 = False                           # exchange q latents in fp8 e4m3

_cache = {}


def _build():
    nc = bacc.Bacc("TRN2", target_bir_lowering=False, debug=False,
                   num_devices=NCORES)
    f32 = mybir.dt.float32
    bf = mybir.dt.bfloat16
    i32 = mybir.dt.int32

    # ---- dram parameters (partition-major packed layouts) ----
    xP = nc.dram_tensor("xP", [P, NKT, SSH], bf, kind="ExternalInput")
    wqaP = nc.dram_tensor("wqaP", [P, NQM, NKT, P], bf, kind="ExternalInput")
    wkvaP = nc.dram_tensor("wkvaP", [P, NKVM, NKT, P], bf,
                           kind="ExternalInput")
    wkpeP = nc.dram_tensor("wkpeP", [P, NKT, ROPE], bf, kind="ExternalInput")
    wqbP = nc.dram_tensor("wqbP", [P, NQM, HPC * QK_HD], bf,
                          kind="ExternalInput")
    wkvbP = nc.dram_tensor("wkvbP", [P, NKVM, HPC * (NOPE + V_DIM)], bf,
                           kind="ExternalInput")
    # wo.T rows regrouped head-major: [p, h, k, dim] = head k*4+h, vdim p
    woP = nc.dram_tensor("woP", [P, HPC, GROUP, DIM], bf,
                         kind="ExternalInput")
    cos_sh = nc.dram_tensor("cos_sh", [P, SSH], bf, kind="ExternalInput")
    sin_sh = nc.dram_tensor("sin_sh", [P, SSH], bf, kind="ExternalInput")
    cos_full = nc.dram_tensor("cos_full", [P, S], bf, kind="ExternalInput")
    sin_full = nc.dram_tensor("sin_full", [P, S], bf, kind="ExternalInput")
    perm64 = nc.dram_tensor("perm64", [P, P], bf, kind="ExternalInput")
    identb = nc.dram_tensor("identb", [P, P], bf, kind="ExternalInput")
    wmask = nc.dram_tensor("wmask", [P, SSH], bf, kind="ExternalInput")
    cfg = nc.dram_tensor("cfg", [1, 3], i32, kind="ExternalInput")
    outT = nc.dram_tensor("out", [DIM, SSH], f32, kind="ExternalOutput")

    qf = mybir.dt.float8e4 if QF8 else bf

    # ---- internal dram ----
    # 8-core RDH AllGathers (shared output) run ~150GB/s on the output
    # size; 4-core groups fall back to the slower Mesh algorithm, so the
    # full 8-core group wins even though half the gather is cross-batch.
    b_kv = nc.dram_tensor("b_kv", [P, BKV_W], bf)
    g_kv = nc.dram_tensor("g_kv", [NCORES * P, BKV_W], bf,
                          addr_space="Shared")
    b_q1 = nc.dram_tensor("b_q1", [P, BQ1_W], qf)
    g_q1 = nc.dram_tensor("g_q1", [NCORES * P, BQ1_W], qf,
                          addr_space="Shared")
    b_q2 = nc.dram_tensor("b_q2", [P, BQ2_W], qf)
    g_q2 = nc.dram_tensor("g_q2", [NCORES * P, BQ2_W], qf,
                          addr_space="Shared")
    o_bnc = [nc.dram_tensor(f"o_bnc{h}", [V_DIM, S], bf) for h in range(HPC)]
    o_g = [nc.dram_tensor(f"o_g{h}", [NCORES * V_DIM, S], bf,
                          addr_space="Shared") for h in range(HPC)]
    rg = [list(range(NCORES))]

    with tile.TileContext(nc) as tc:
        with (
            tc.tile_pool(name="persist", bufs=1) as persist,
            tc.tile_pool(name="attn", bufs=1) as attn_pool,
            tc.tile_pool(name="wts", bufs=1) as wts,
        ):
            # ---- early big DMAs first ----
            # weights/tables on the scalar queue (idle early)
            wkvb = wts.tile([P, NKVM, HPC * (NOPE + V_DIM)], bf)
            nc.scalar.dma_start(out=wkvb, in_=wkvbP[:])
            mask_sb = persist.tile([P, SSH], bf)
            nc.scalar.dma_start(out=mask_sb, in_=wmask[:])
            ident_sb = persist.tile([P, P], bf)
            nc.scalar.dma_start(out=ident_sb, in_=identb[:])
            cos_f_sb = persist.tile([P, S], bf)
            nc.scalar.dma_start(out=cos_f_sb, in_=cos_full[:])
            sin_f_sb = persist.tile([P, S], bf)
            nc.scalar.dma_start(out=sin_f_sb, in_=sin_full[:])
            perm_sb = persist.tile([P, P], bf)
            nc.scalar.dma_start(out=perm_sb, in_=perm64[:])
            cos_sh_sb = persist.tile([P, SSH], bf)
            nc.scalar.dma_start(out=cos_sh_sb, in_=cos_sh[:])
            sin_sh_sb = persist.tile([P, SSH], bf)
            nc.scalar.dma_start(out=sin_sh_sb, in_=sin_sh[:])

            # constants (cheap, engine-local)
            ones_b = persist.tile([P, 1], bf)
            nc.vector.memset(ones_b, 1.0)
            eps_sb = persist.tile([1, 1], f32)
            nc.vector.memset(eps_sb, EPS)
            cfg_sb = persist.tile([1, 3], i32)
            nc.gpsimd.dma_start(out=cfg_sb, in_=cfg[:])

            # wqb in its own pool so it can be freed after q-up
            wqb_cm = tc.tile_pool(name="wqbp", bufs=1)
            wqb_pool = wqb_cm.__enter__()
            wqb = wqb_pool.tile([P, NQM, HPC * QK_HD], bf)
            nc.scalar.dma_start(out=wqb, in_=wqbP[:])

            # attention-phase persistent tiles (filled by up-proj)
            qt_nope = [attn_pool.tile([P, S], bf, tag=f"qtn{h}",
                                      name=f"qt_nope{h}") for h in range(HPC)]
            qt_pe = [attn_pool.tile([P, S], bf, tag=f"qtp{h}",
                                    name=f"qt_pe{h}")
                     for h in range(HPC // 2)]
            kt_nope = [attn_pool.tile([P, S], bf, tag=f"ktn{h}",
                                      name=f"kt_nope{h}") for h in range(HPC)]
            v_all = attn_pool.tile([P, S // P, HPC * V_DIM], bf)
            kpe_dup = attn_pool.tile([P, NCH, SSH], bf)

            # ======== Phase A + up-proj (shared latent pool) ========
            up_lat_cm = tc.tile_pool(name="up_lat", bufs=1)
            up_lat = up_lat_cm.__enter__()
            with (
                tc.tile_pool(name="pa", bufs=2) as pa,
                tc.tile_pool(name="pa_x", bufs=1) as pa_x,
                tc.tile_pool(name="pa_out", bufs=3) as pa_out,
                tc.tile_pool(name="pa_ps", bufs=2, space="PSUM") as pa_ps,
                tc.tile_pool(name="pa_st", bufs=1, space="PSUM") as pa_st,
            ):
                x_all = pa_x.tile([P, NKT, SSH], bf)
                nc.sync.dma_start(out=x_all[:, 0:NKT // 2, :],
                                  in_=xP[:, 0:NKT // 2, :])
                nc.sync.dma_start(out=x_all[:, NKT // 2:, :],
                                  in_=xP[:, NKT // 2:, :])

                q_stat = pa_st.tile([1, SSH], f32)
                kv_stat = pa_st.tile([1, SSH], f32)

                def down_slab(wP_m, mrows, bounce, dst_col, stat_ps,
                              stat_first, stat_last, ev_tag="ev", ev_dt=bf):
                    slab = pa.tile([P, NKT, mrows], bf, tag="slab")
                    nc.sync.dma_start(out=slab, in_=wP_m)
                    ps = pa_ps.tile([P, SSH], f32, tag="dps")
                    for k in range(NKT):
                        nc.tensor.matmul(ps[:mrows, :], slab[:, k, :],
                                         x_all[:, k, :], start=(k == 0),
                                         stop=(k == NKT - 1))
                    ev = pa_out.tile([P, SSH], ev_dt, tag=ev_tag)
                    nc.vector.tensor_copy(ev[:mrows, :], ps[:mrows, :])
                    if bounce is not None:
                        nc.sync.dma_start(
                            out=bounce[:, dst_col:dst_col + SSH],
                            in_=ev[:mrows, :])
                    if stat_ps is not None:
                        sq = pa.tile([P, SSH], bf, tag="sq")
                        nc.scalar.square(sq[:mrows, :], ps[:mrows, :])
                        nc.tensor.matmul(stat_ps, ones_b[:mrows, :],
                                         sq[:mrows, :], start=stat_first,
                                         stop=stat_last)
                    return ev

                def stat_row(stat, n, dst, broadcast):
                    tmp = pa.tile([1, SSH], f32, tag="srt")
                    nc.scalar.activation(tmp, stat,
                                         mybir.ActivationFunctionType.Sqrt,
                                         bias=eps_sb[0:1, 0:1], scale=1.0 / n)
                    rcp = pa.tile([1, SSH], f32, tag="rcp")
                    nc.vector.reciprocal(rcp, tmp)
                    rb = pa.tile([1, SSH], bf, tag="rb")
                    nc.vector.tensor_copy(rb, rcp)
                    if broadcast:
                        bc = pa.tile([P, SSH], bf, tag="sbc")
                        nc.gpsimd.partition_broadcast(bc, rb)
                        nc.sync.dma_start(out=dst, in_=bc)
                    else:
                        nc.sync.dma_start(out=dst, in_=rb)

                # ---- kv first (so AG_kv overlaps q down-proj) ----
                for m in range(NKVM):
                    down_slab(wkvaP[:, m], P, b_kv, m * SSH, kv_stat,
                              m == 0, m == NKVM - 1)
                kpe_ev = down_slab(wkpeP[:], ROPE, None, None, None,
                                   False, False, ev_tag="kpe_ev")
                xs_ps = pa_ps.tile([ROPE, SSH], f32, tag="xs")
                nc.tensor.matmul(xs_ps, perm_sb[:ROPE, :ROPE], kpe_ev[:ROPE, :])
                y0 = pa.tile([ROPE, SSH], bf, tag="ry0")
                nc.vector.tensor_mul(y0, kpe_ev[:ROPE, :], cos_sh_sb[:ROPE, :])
                y1 = pa.tile([ROPE, SSH], bf, tag="ry1")
                nc.vector.tensor_mul(y1, xs_ps, sin_sh_sb[:ROPE, :])
                yr = pa.tile([ROPE, SSH], bf, tag="ryr")
                nc.vector.tensor_add(yr, y0, y1)
                nc.sync.dma_start(out=b_kv[0:ROPE, NKVM * SSH:
                                           (NKVM + 1) * SSH], in_=yr)
                nc.sync.dma_start(out=b_kv[ROPE:P, NKVM * SSH:
                                           (NKVM + 1) * SSH], in_=yr)
                stat_row(kv_stat, KV_LORA, b_kv[:, (NKVM + 1) * SSH:],
                         broadcast=True)
                nc.gpsimd.collective_compute(
                    "AllGather", mybir.AluOpType.bypass, replica_groups=rg,
                    ins=[b_kv[:]], outs=[g_kv[:]])

                # per-core dynamic offsets (needed only from gathers on)
                r0 = nc.alloc_registers()
                nc.regs_load(r0, cfg_sb[0:1, 0:1])
                rb_base = nc.snap(r0, donate=True, min_val=0,
                                  max_val=(NCORES - GROUP) * P)
                r1 = nc.alloc_registers()
                nc.regs_load(r1, cfg_sb[0:1, 1:2])
                rg_col = nc.snap(r1, donate=True, min_val=0, max_val=S - SSH)

                # ---- q down-proj part 1 (overlaps AG_kv) ----
                for m in range(NQ1):
                    down_slab(wqaP[:, m], P, b_q1, m * SSH, q_stat,
                              m == 0, False, ev_dt=qf)
                nc.gpsimd.collective_compute(
                    "AllGather", mybir.AluOpType.bypass, replica_groups=rg,
                    ins=[b_q1[:]], outs=[g_q1[:]])

                # kv gather loads on the scalar queue (prefetches done;
                # these wait on AG_kv without blocking q2 slab DMAs)
                kv_lat = up_lat.tile([P, NCH, NKVM, SSH], bf)
                a_kv_bc = up_lat.tile([P, NCH, SSH], bf)
                for r in range(NCH):
                    row = bass.ds(rb_base + r * P, P)
                    nc.scalar.dma_start(
                        out=kv_lat[:, r],
                        in_=g_kv[row, 0:NKVM * SSH].rearrange(
                            "p (k s) -> p k s", s=SSH))
                    nc.scalar.dma_start(
                        out=kpe_dup[:, r, :],
                        in_=g_kv[row, NKVM * SSH:(NKVM + 1) * SSH])
                    nc.scalar.dma_start(
                        out=a_kv_bc[:, r, :],
                        in_=g_kv[row, (NKVM + 1) * SSH:])

                # ---- q down-proj part 2 (overlaps AG_kv / AG_q1) ----
                for m in range(NQ1, NQM):
                    down_slab(wqaP[:, m], P, b_q2, (m - NQ1) * SSH, q_stat,
                              False, m == NQM - 1, ev_dt=qf)
                stat_row(q_stat, Q_LORA, b_q2[:, NQ2 * SSH:],
                         broadcast=True)
                nc.gpsimd.collective_compute(
                    "AllGather", mybir.AluOpType.bypass, replica_groups=rg,
                    ins=[b_q2[:]], outs=[g_q2[:]])

                # a_q gather on scalar queue
                a_q_bc = up_lat.tile([P, NCH, SSH], bf)
                for r in range(NCH):
                    nc.scalar.dma_start(
                        out=a_q_bc[:, r, :],
                        in_=g_q2[bass.ds(rb_base + r * P, P), NQ2 * SSH:])

            # ================= Phase B: up projections =================
            with (
                tc.tile_pool(name="up", bufs=3) as up,
                tc.tile_pool(name="qlat", bufs=2) as qlat_pool,
                tc.tile_pool(name="up_ps", bufs=3, space="PSUM") as up_ps,
                tc.tile_pool(name="pe_ps", bufs=2, space="PSUM") as pe_ps,
            ):
                # prescale kv latent by inv-rms (q scaled at eviction)
                for k in range(NKVM):
                    for r in range(NCH):
                        nc.vector.tensor_mul(kv_lat[:, r, k, :],
                                             kv_lat[:, r, k, :],
                                             a_kv_bc[:, r, :])
                # k_nope (d-major) + v (row-major)
                for c in range(NCH):
                    for h in range(HPC):
                        ps = up_ps.tile([P, SSH], f32, tag="up")
                        for k in range(NKVM):
                            nc.tensor.matmul(
                                ps, wkvb[:, k, h * NOPE:(h + 1) * NOPE],
                                kv_lat[:, c, k, :], start=(k == 0),
                                stop=(k == NKVM - 1))
                        nc.vector.tensor_copy(
                            kt_nope[h][:, c * SSH:(c + 1) * SSH], ps)
                for sb in range(S // P):
                    c, part = sb // 4, sb % 4
                    ps = up_ps.tile([P, HPC * V_DIM], f32, tag="up")
                    for k in range(NKVM):
                        nc.tensor.matmul(
                            ps, kv_lat[:, c, k, part * P:(part + 1) * P],
                            wkvb[:, k, HPC * NOPE:], start=(k == 0),
                            stop=(k == NKVM - 1))
                    nc.vector.tensor_copy(v_all[:, sb, :], ps)

                # ---- q up-proj (waits on AG_q1/2; q_lat streamed) ----
                for c in range(NCH):
                    qlf = qlat_pool.tile([P, NQM, SSH], qf, tag="qlf",
                                         name="qlf")
                    nc.sync.dma_start(
                        out=qlf[:, 0:NQ1, :],
                        in_=g_q1[bass.ds(rb_base + c * P, P), :].rearrange(
                            "p (m s) -> p m s", s=SSH))
                    nc.sync.dma_start(
                        out=qlf[:, NQ1:, :],
                        in_=g_q2[bass.ds(rb_base + c * P, P),
                                 0:NQ2 * SSH].rearrange(
                            "p (m s) -> p m s", s=SSH))
                    if QF8:
                        ql = qlat_pool.tile([P, NQM, SSH], bf, tag="ql",
                                            name="ql")
                        nc.vector.tensor_copy(ql[:, 0:NQ1, :],
                                              qlf[:, 0:NQ1, :])
                        nc.vector.tensor_copy(ql[:, NQ1:, :],
                                              qlf[:, NQ1:, :])
                    else:
                        ql = qlf
                    for h in range(HPC):
                        ps = up_ps.tile([P, SSH], f32, tag="up")
                        for k in range(NQM):
                            nc.tensor.matmul(
                                ps, wqb[:, k, h * P:(h + 1) * P],
                                ql[:, k, :], start=(k == 0),
                                stop=(k == NQM - 1))
                        nc.vector.tensor_mul(
                            qt_nope[h][:, c * SSH:(c + 1) * SSH], ps,
                            a_q_bc[:, c, :])
                    for hp in range(HPC // 2):
                        pcol = HPC * NOPE + 2 * hp * ROPE
                        ps = pe_ps.tile([P, SSH], f32, tag="qp")
                        for k in range(NQM):
                            nc.tensor.matmul(
                                ps, wqb[:, k, pcol:pcol + 2 * ROPE],
                                ql[:, k, :], start=(k == 0),
                                stop=(k == NQM - 1))
                        pe_s = up.tile([P, SSH], bf, tag="pes")
                        nc.vector.tensor_mul(pe_s, ps, a_q_bc[:, c, :])
                        xs = pe_ps.tile([P, SSH], f32, tag="qpx")
                        nc.tensor.matmul(xs, perm_sb, pe_s)
                        dst = qt_pe[hp][:, c * SSH:(c + 1) * SSH]
                        nc.vector.tensor_mul(
                            dst, pe_s, cos_f_sb[:, c * SSH:(c + 1) * SSH])
                        t1 = up.tile([P, SSH], bf, tag="pet")
                        nc.vector.tensor_mul(
                            t1, xs, sin_f_sb[:, c * SSH:(c + 1) * SSH])
                        nc.vector.tensor_add(dst, dst, t1)

            up_lat_cm.__exit__(None, None, None)
            wqb_cm.__exit__(None, None, None)

            # ========== attention + per-head AGs + interleaved wo ==========
            with (
                tc.tile_pool(name="at", bufs=3) as at,
                tc.tile_pool(name="at_ps", bufs=2) as at_psum_sb,
                tc.tile_pool(name="at_rl", bufs=2) as at_rl,
                tc.tile_pool(name="wo_rhs", bufs=2) as wo_rhs,
                tc.tile_pool(name="wo_acc", bufs=1) as wo_acc,
                tc.tile_pool(name="wo_w", bufs=2) as wo_w,
                tc.tile_pool(name="wo_ev", bufs=3) as wo_ev,
                tc.tile_pool(name="st_ps", bufs=3, space="PSUM") as st_ps,
                tc.tile_pool(name="ot_ps", bufs=2, space="PSUM") as ot_ps,
                tc.tile_pool(name="l_ps", bufs=1, space="PSUM") as l_ps,
                tc.tile_pool(name="wo_ps", bufs=2, space="PSUM") as wo_ps,
            ):
                acc = wo_acc.tile([P, NKT, SSH], f32)

                def attention_head(h):
                    pend = []        # [(pj, off, first, last, ot, j), ...]
                    pend_ev = None   # (ot, P_sum, qc)

                    def flush2(keep=0):
                        while len(pend) > keep:
                            pj, off, first, last, ot, j = pend.pop(0)
                            nc.tensor.matmul(
                                ot[:, off:],
                                v_all[:, j, h * V_DIM:(h + 1) * V_DIM],
                                pj[:, off:], start=first, stop=last)

                    def evict():
                        nonlocal pend_ev
                        if pend_ev is None:
                            return
                        ot, psum, qc = pend_ev
                        lt = l_ps.tile([1, SSH], f32, tag="l", name="lt")
                        nc.tensor.matmul(lt, ones_b, psum)
                        rinv = at_rl.tile([1, SSH], f32, tag="ri",
                                          name="rinv")
                        nc.vector.reciprocal_approx_fast(rinv, lt)
                        rlb = at_rl.tile([P, SSH], f32, tag="rlb",
                                         name="rlb")
                        nc.gpsimd.partition_broadcast(rlb, rinv)
                        ev = at.tile([P, SSH], bf, tag="oev", name="oev")
                        nc.vector.tensor_mul(ev, ot, rlb)
                        nc.sync.dma_start(
                            out=o_bnc[h][:, qc * SSH:(qc + 1) * SSH],
                            in_=ev)
                        pend_ev = None

                    for qc in range(NCH):
                        nj = qc * 4 + 4
                        ot = ot_ps.tile([P, SSH], f32, tag="ot", name="ot")
                        psum = at_psum_sb.tile([P, SSH], bf, tag="ps",
                                               name="psum")
                        for j in range(nj):
                            d = j - qc * 4
                            off = max(0, d) * P
                            st = st_ps.tile([P, SSH], f32, tag="st",
                                            name="st")
                            nc.tensor.matmul(
                                st[:, off:],
                                kt_nope[h][:, j * P:(j + 1) * P],
                                qt_nope[h][:, qc * SSH + off:(qc + 1) * SSH],
                                start=True, stop=False)
                            lo = (h % 2) * ROPE
                            nc.tensor.matmul(
                                st[:, off:],
                                kpe_dup[lo:lo + ROPE, j // 4,
                                        (j % 4) * P:(j % 4 + 1) * P],
                                qt_pe[h // 2][lo:lo + ROPE,
                                              qc * SSH + off:(qc + 1) * SSH],
                                start=False, stop=(d < 0))
                            if d >= 0:
                                # causal mask folded into the PSUM group
                                nc.tensor.matmul(
                                    st[:, off:], ident_sb,
                                    mask_sb[:, 0:SSH - off],
                                    start=False, stop=True)
                            flush2(keep=1)
                            if j == 1:
                                evict()  # previous qc, off the exp path
                            pj = at.tile([P, SSH], bf, tag="p", name="pj")
                            nc.scalar.activation(
                                pj[:, off:], st[:, off:],
                                mybir.ActivationFunctionType.Exp)
                            if j == 0:
                                nc.vector.tensor_copy(psum, pj)
                            else:
                                nc.vector.tensor_add(psum[:, off:],
                                                     psum[:, off:],
                                                     pj[:, off:])
                            pend.append((pj, off, j == 0, j == nj - 1,
                                         ot, j))
                        flush2()
                        pend_ev = (ot, psum, qc)
                    evict()
                    nc.gpsimd.collective_compute(
                        "AllGather", mybir.AluOpType.bypass,
                        replica_groups=rg, ins=[o_bnc[h][:]],
                        outs=[o_g[h][:]])
                    # prefetch this head's wo weights (no deps) first, then
                    # the rhs loads — both on the scalar queue so the AG
                    # trigger chain on gpsimd is never blocked behind DMAs
                    wslab = wo_w.tile([P, GROUP, DIM], bf, tag="woslab",
                                      name="wslab")
                    nc.scalar.dma_start(out=wslab, in_=woP[:, h])
                    rhs = wo_rhs.tile([P, GROUP, SSH], bf, tag="rhs",
                                      name="rhs")
                    for k in range(GROUP):
                        nc.scalar.dma_start(
                            out=rhs[:, k, :],
                            in_=o_g[h][bass.ds(rb_base + k * P, P),
                                       bass.ds(rg_col, SSH)])
                    return rhs, wslab

                def wo_pass(h, rhs, wslab):
                    for m in range(NKT):
                        ps = wo_ps.tile([P, SSH], f32, tag="wops",
                                        name="wops")
                        for k in range(GROUP):
                            nc.tensor.matmul(
                                ps, wslab[:, k, m * P:(m + 1) * P],
                                rhs[:, k, :], start=(k == 0),
                                stop=(k == GROUP - 1))
                        if h == 0:
                            nc.vector.tensor_copy(acc[:, m, :], ps)
                        elif h < HPC - 1:
                            nc.vector.tensor_add(acc[:, m, :], ps,
                                                 acc[:, m, :])
                        else:
                            ev = wo_ev.tile([P, SSH], f32, tag="woev",
                                            name="woev")
                            nc.vector.tensor_add(ev, ps, acc[:, m, :])
                            nc.sync.dma_start(out=outT[m * P:(m + 1) * P, :],
                                              in_=ev)

                heads_rhs = {}
                for h in range(HPC):
                    heads_rhs[h] = attention_head(h)
                    if h >= 1:
                        wo_pass(h - 1, *heads_rhs[h - 1])
                wo_pass(HPC - 1, *heads_rhs[HPC - 1])

    nc.compile()
    return nc


def _prep_inputs(x, freqs_cos, freqs_sin, wq_a, q_norm_w, wq_b, wkv_a,
                 kv_norm_w, wkv_b, wo):
    x = np.asarray(x, np.float32)
    freqs_cos = np.asarray(freqs_cos, np.float32)
    freqs_sin = np.asarray(freqs_sin, np.float32)
    wq_a = np.asarray(wq_a, np.float32)
    q_norm_w = np.asarray(q_norm_w, np.float32)
    wq_b = np.asarray(wq_b, np.float32)
    wkv_a = np.asarray(wkv_a, np.float32)
    kv_norm_w = np.asarray(kv_norm_w, np.float32)
    wkv_b = np.asarray(wkv_b, np.float32)
    wo = np.asarray(wo, np.float32)

    wqaT = np.ascontiguousarray(wq_a.T)          # [DIM, Q_LORA]
    wkvaT = np.ascontiguousarray(wkv_a.T)        # [DIM, KV_LORA+ROPE]
    # partition-major packs: [p, m, kt, j] etc.
    wqaP = np.ascontiguousarray(
        wqaT.reshape(NKT, P, NQM, P).transpose(1, 2, 0, 3)).astype(BF16)
    wkvaP = np.ascontiguousarray(
        wkvaT[:, :KV_LORA].reshape(NKT, P, NKVM, P)
        .transpose(1, 2, 0, 3)).astype(BF16)
    wkpeP = np.ascontiguousarray(
        wkvaT[:, KV_LORA:].reshape(NKT, P, ROPE)
        .transpose(1, 0, 2)).astype(BF16)

    wqb_eff = (wq_b * q_norm_w[None, :]) * SCALE
    wqb_eff = wqb_eff.reshape(N_HEADS, QK_HD, Q_LORA)
    wkvb_eff = wkv_b * kv_norm_w[None, :]
    wkvb_eff = wkvb_eff.reshape(N_HEADS, NOPE + V_DIM, KV_LORA)

    cosT = np.tile(np.repeat(freqs_cos.T, 2, axis=0), (2, 1))  # [128, S]
    sinT = np.tile(np.repeat(freqs_sin.T, 2, axis=0), (2, 1))

    perm64_ = np.zeros((ROPE, ROPE), np.float32)
    for i in range(ROPE // 2):
        perm64_[2 * i + 1, 2 * i] = -1.0  # out[2i]   = -x[2i+1]
        perm64_[2 * i, 2 * i + 1] = 1.0   # out[2i+1] =  x[2i]
    perm = np.zeros((P, P), np.float32)
    perm[:ROPE, :ROPE] = perm64_
    perm[ROPE:, ROPE:] = perm64_
    r = np.arange(P)
    # wide causal mask for the diagonal score blocks: [kv r, q t]
    wmask = np.zeros((P, SSH), np.float32)
    wmask[:, :P] = np.where(r[:, None] <= r[None, :], 0.0, -1e30)
    ident = np.eye(P, dtype=np.float32)

    # wo.T rows regrouped so pass h contracts head k*4+h for k=0..3:
    # woP[p, h, k, :] = wo.T row of head k*4+h, vdim p
    woT4 = wo.T.reshape(N_HEADS // 4, 4, V_DIM, DIM)  # [k, h, p, D]
    woP = np.ascontiguousarray(woT4.transpose(2, 1, 0, 3)).astype(BF16)

    in_maps = []
    for c in range(NCORES):
        b, g = c // GROUP, c % GROUP
        heads = slice(g * HPC, (g + 1) * HPC)
        xTc = np.ascontiguousarray(x[b].T[:, g * SSH:(g + 1) * SSH])
        xPc = np.ascontiguousarray(
            xTc.reshape(NKT, P, SSH).transpose(1, 0, 2)).astype(BF16)
        wqbT = np.concatenate(
            [wqb_eff[heads, :NOPE].reshape(HPC * NOPE, Q_LORA),
             wqb_eff[heads, NOPE:].reshape(HPC * ROPE, Q_LORA)],
            axis=0).T                                  # [Q_LORA, 768]
        wkvbT = np.concatenate(
            [wkvb_eff[heads, :NOPE].reshape(HPC * NOPE, KV_LORA),
             wkvb_eff[heads, NOPE:].reshape(HPC * V_DIM, KV_LORA)],
            axis=0).T                                  # [KV_LORA, 1024]
        wqbP = np.ascontiguousarray(
            wqbT.reshape(NQM, P, HPC * QK_HD).transpose(1, 0, 2)).astype(BF16)
        wkvbP = np.ascontiguousarray(
            wkvbT.reshape(NKVM, P, HPC * (NOPE + V_DIM))
            .transpose(1, 0, 2)).astype(BF16)
        in_maps.append({
            "xP": xPc,
            "wqaP": wqaP,
            "wkvaP": wkvaP,
            "wkpeP": wkpeP,
            "wqbP": wqbP,
            "wkvbP": wkvbP,
            "woP": woP,
            "cos_sh": np.ascontiguousarray(
                cosT[:, g * SSH:(g + 1) * SSH]).astype(BF16),
            "sin_sh": np.ascontiguousarray(
                sinT[:, g * SSH:(g + 1) * SSH]).astype(BF16),
            "cos_full": np.ascontiguousarray(cosT).astype(BF16),
            "sin_full": np.ascontiguousarray(sinT).astype(BF16),
            "perm64": perm.astype(BF16),
            "identb": ident.astype(BF16),
            "wmask": wmask.astype(BF16),
            "cfg": np.array([[b * GROUP * P, g * SSH, b * GROUP]],
                            np.int32),
        })
    return in_maps


def _run(inputs, trace=False, **kw):
    if "nc" not in _cache:
        _cache["nc"] = _build()
    nc = _cache["nc"]
    in_maps = _prep_inputs(**inputs)
    res = run_bass_kernel_spmd(nc, in_maps, core_ids=list(range(NCORES)),
                               trace=trace, **kw)
    out = np.empty((B, S, DIM), np.float32)
    for c in range(NCORES):
        b, g = c // GROUP, c % GROUP
        out[b, g * SSH:(g + 1) * SSH, :] = res.results[c]["out"].T
    return out, res


def kernel(**inputs):
    out, _ = _run(inputs)
    return out

